# revision 10
# baseline (speedup 1.0000x reference)
"""Trainium2 Bass kernel for EquivariantBallUnpooling.

Computation (per parent node n with stride=8 children):
  rel   = child_pos.reshape(n,8,3) - pos[:,None,:]
  cat_mv = [x_mv (16ch) ; embed_translation(rel) (8ch)]        # [n,24,8]
  cat_s  = [x_s (32) ; ||rel|| (8)]                            # [n,40]
  y_mv[n,o,i] = sum_c cat_mv[n,c,i] * W_mv[grade(i)][o,c]      # o in 0..127
  y_s = cat_s @ W_s.T + b_s                                    # [n,256]
  out_mv = child_x_mv + y_mv.reshape(n*8,16,8)
  out_s  = child_x_s  + y_s.reshape(n*8,32)

Sharding: data-parallel over parents across 8 cores; children stay with
their parent (blocks of stride). Weights replicated.

Per-core device kernel layout strategy: parents on the 128 SBUF
partitions, features on the free dim. Each parent's 8 children are
contiguous rows in DRAM, so child tensors load as [128, 8*feat]
contiguous lines. Activations are transposed on-chip via the PE
(contraction dim -> partitions) and the grade-wise equivariant linear
becomes 11 small matmuls + 1 for the scalar channel (bias folded in as
an extra constant-1 row).
"""

import numpy as np

import concourse.bass as bass
import concourse.mybir as mybir
import concourse.tile as tile
from concourse import bacc
from concourse.bass_utils import run_bass_kernel_spmd
from concourse.masks import make_identity

F32 = mybir.dt.float32
N_CORES = 8
STRIDE = 8
N_TOTAL = 65536
N_PER_CORE = N_TOTAL // N_CORES  # 8192
GRADE = [0, 1, 1, 1, 2, 2, 2, 3]
P = 128


def build_nc(npc: int) -> bass.Bass:
    """Build the per-core Bass program for npc parents."""
    assert npc % P == 0
    ntiles = npc // P

    nc = bacc.Bacc("TRN2", target_bir_lowering=False)

    x = nc.dram_tensor("x", [npc, 128], F32, kind="ExternalInput")
    xs = nc.dram_tensor("xs", [npc, 32], F32, kind="ExternalInput")
    pos = nc.dram_tensor("pos", [npc, 3], F32, kind="ExternalInput")
    cpos = nc.dram_tensor("cpos", [npc, 24], F32, kind="ExternalInput")
    cmv = nc.dram_tensor("cmv", [npc, 1024], F32, kind="ExternalInput")
    cs = nc.dram_tensor("cs", [npc, 256], F32, kind="ExternalInput")
    w1 = nc.dram_tensor("w1", [128, 1024], F32, kind="ExternalInput")
    w2 = nc.dram_tensor("w2", [24, 384], F32, kind="ExternalInput")
    ws = nc.dram_tensor("ws", [41, 256], F32, kind="ExternalInput")
    omv = nc.dram_tensor("omv", [npc, 1024], F32, kind="ExternalOutput")
    osc = nc.dram_tensor("os", [npc, 256], F32, kind="ExternalOutput")

    with tile.TileContext(nc) as tc:
        with (
            tc.tile_pool(name="singles", bufs=1) as singles,
            tc.tile_pool(name="loads", bufs=3) as loads,
            tc.tile_pool(name="work", bufs=3) as work,
            tc.tile_pool(name="tp_psum", bufs=2, space="PSUM") as tp_psum,
            tc.tile_pool(name="mv_psum", bufs=2, space="PSUM") as mv_psum,
            tc.tile_pool(name="s_psum", bufs=2, space="PSUM") as s_psum,
        ):
            ident = singles.tile([P, P], F32)
            make_identity(nc, ident[:])
            w1_sb = singles.tile([128, 1024], F32)
            nc.sync.dma_start(out=w1_sb[:], in_=w1[:, :])
            w2_sb = singles.tile([24, 384], F32)
            nc.sync.dma_start(out=w2_sb[:], in_=w2[:, :])
            ws_sb = singles.tile([41, 256], F32)
            nc.sync.dma_start(out=ws_sb[:], in_=ws[:, :])

            for t in range(ntiles):
                r0 = t * P

                x_t = loads.tile([P, 128], F32)
                nc.sync.dma_start(out=x_t[:], in_=x[r0 : r0 + P])
                pos_t = loads.tile([P, 3], F32)
                nc.sync.dma_start(out=pos_t[:], in_=pos[r0 : r0 + P])
                cpos_t = loads.tile([P, 24], F32)
                nc.sync.dma_start(out=cpos_t[:], in_=cpos[r0 : r0 + P])
                cmv_t = loads.tile([P, 1024], F32)
                nc.sync.dma_start(out=cmv_t[:], in_=cmv[r0 : r0 + P])
                cs_t = loads.tile([P, 256], F32)
                nc.sync.dma_start(out=cs_t[:], in_=cs[r0 : r0 + P])
                cats = work.tile([P, 41], F32)
                nc.sync.dma_start(out=cats[:, 0:32], in_=xs[r0 : r0 + P])

                # rel[p, m, d] = cpos[p, m, d] - pos[p, d]
                rel_t = work.tile([P, 24], F32)
                rel_v = rel_t[:].rearrange("p (m d) -> p m d", d=3)
                cpos_v = cpos_t[:].rearrange("p (m d) -> p m d", d=3)
                for d in range(3):
                    nc.vector.tensor_scalar_sub(
                        out=rel_v[:, :, d],
                        in0=cpos_v[:, :, d],
                        scalar1=pos_t[:, d : d + 1],
                    )
                # ||rel||: square, reduce over d, sqrt into cat_s cols 32:40
                sq_t = work.tile([P, 24], F32)
                nc.vector.tensor_mul(sq_t[:], rel_t[:], rel_t[:])
                nrm2_t = work.tile([P, 8], F32)
                nc.vector.reduce_sum(
                    out=nrm2_t[:],
                    in_=sq_t[:].rearrange("p (m d) -> p m d", d=3),
                    axis=mybir.AxisListType.X,
                )
                nc.scalar.activation(
                    cats[:, 32:40], nrm2_t[:], mybir.ActivationFunctionType.Sqrt
                )
                nc.vector.memset(cats[:, 40:41], 1.0)

                # Transposes: contraction dims onto partitions, natural row
                # order (xT rows = c*8+i, relT rows = m*3+d); the matching
                # row permutation is baked into the host-side weight layout.
                xt_ps = tp_psum.tile([P, P], F32, tag="tp")
                nc.tensor.transpose(xt_ps[:], x_t[:], ident[:])
                xT_sb = work.tile([P, P], F32)
                nc.scalar.copy(xT_sb[:], xt_ps[:])

                relT_ps = tp_psum.tile([24, P], F32, tag="tp")
                nc.tensor.transpose(relT_ps[:], rel_t[:], ident[:])
                relT_sb = work.tile([24, P], F32)
                nc.vector.tensor_copy(relT_sb[:], relT_ps[:])

                catsT_ps = tp_psum.tile([41, P], F32, tag="tp")
                nc.tensor.transpose(catsT_ps[:], cats[:], ident[:])
                catsT_sb = work.tile([41, P], F32)
                nc.vector.tensor_copy(catsT_sb[:], catsT_ps[:])

                # Grade-wise equivariant linear. Weights are block-diagonal
                # zero-padded so every matmul contracts the full transposed
                # tile from partition 0 (HW: operand base must be 0/32/64)
                # and the stationary operand is shared across components.
                mv_ps = mv_psum.tile([P, 1024], F32)
                for i in range(8):
                    nc.tensor.matmul(
                        mv_ps[:, i * 128 : (i + 1) * 128],
                        lhsT=xT_sb[:],
                        rhs=w1_sb[:, i * 128 : (i + 1) * 128],
                        start=True,
                        stop=i not in (1, 2, 3),
                    )
                    if i in (1, 2, 3):
                        d = i - 1
                        nc.tensor.matmul(
                            mv_ps[:, i * 128 : (i + 1) * 128],
                            lhsT=relT_sb[:],
                            rhs=w2_sb[:, d * 128 : (d + 1) * 128],
                            start=False,
                            stop=True,
                        )

                s_ps = s_psum.tile([P, 256], F32)
                nc.tensor.matmul(
                    s_ps[:], lhsT=catsT_sb[:], rhs=ws_sb[:], start=True, stop=True
                )

                # out = child + y.  mv psum holds [i, o] slabs; child rows
                # are [o, i] interleaved, so read psum with a permuted AP.
                nc.vector.tensor_add(
                    out=cmv_t[:].rearrange("p (o i) -> p o i", i=8),
                    in0=cmv_t[:].rearrange("p (o i) -> p o i", i=8),
                    in1=mv_ps[:].rearrange("p (i o) -> p o i", i=8),
                )
                nc.vector.tensor_add(out=cs_t[:], in0=cs_t[:], in1=s_ps[:])

                nc.scalar.dma_start(out=omv[r0 : r0 + P], in_=cmv_t[:])
                nc.scalar.dma_start(out=osc[r0 : r0 + P], in_=cs_t[:])

    nc.compile()
    return nc


def _prep_weights(W_mv: np.ndarray, W_s: np.ndarray, b_s: np.ndarray):
    # Block-diagonal layouts matching the on-chip transposed activations:
    # xT rows are (c*8 + i), relT rows are (m*3 + d).
    w1 = np.zeros((128, 1024), np.float32)
    w1r = w1.reshape(16, 8, 8, 128)  # [c, i_row, i_slab, o]
    w2 = np.zeros((24, 384), np.float32)
    w2r = w2.reshape(8, 3, 3, 128)  # [m, d_row, d_slab, o]
    for i in range(8):
        w1r[:, i, i, :] = W_mv[GRADE[i]][:, :16].T
    for d in range(3):
        w2r[:, d, d, :] = W_mv[GRADE[d + 1]][:, 16:24].T
    ws = np.concatenate([W_s.T, b_s[None, :]], axis=0).astype(np.float32)
    return w1, w2, ws


_NC_CACHE: dict[int, bass.Bass] = {}


def _get_nc(npc: int) -> bass.Bass:
    if npc not in _NC_CACHE:
        _NC_CACHE[npc] = build_nc(npc)
    return _NC_CACHE[npc]


def kernel(**inputs) -> tuple[np.ndarray, np.ndarray]:
    x_mv = np.ascontiguousarray(np.asarray(inputs["x_mv"], np.float32))
    x_s = np.ascontiguousarray(np.asarray(inputs["x_s"], np.float32))
    pos = np.ascontiguousarray(np.asarray(inputs["pos"], np.float32))
    child_x_mv = np.ascontiguousarray(np.asarray(inputs["child_x_mv"], np.float32))
    child_x_s = np.ascontiguousarray(np.asarray(inputs["child_x_s"], np.float32))
    child_pos = np.ascontiguousarray(np.asarray(inputs["child_pos"], np.float32))
    W_mv = np.asarray(inputs["W_mv"], np.float32)
    W_s = np.asarray(inputs["W_s"], np.float32)
    b_s = np.asarray(inputs["b_s"], np.float32)
    stride = int(np.asarray(inputs["stride"]))
    assert stride == STRIDE, stride

    n = x_mv.shape[0]
    npc = n // N_CORES
    nc = _get_nc(npc)
    w1, w2, ws = _prep_weights(W_mv, W_s, b_s)

    in_maps = []
    for c in range(N_CORES):
        rp = slice(c * npc, (c + 1) * npc)
        rc = slice(c * npc * STRIDE, (c + 1) * npc * STRIDE)
        in_maps.append(
            {
                "x": x_mv[rp].reshape(npc, 128),
                "xs": x_s[rp],
                "pos": pos[rp],
                "cpos": child_pos[rc].reshape(npc, 24),
                "cmv": child_x_mv[rc].reshape(npc, 1024),
                "cs": child_x_s[rc].reshape(npc, 256),
                "w1": w1,
                "w2": w2,
                "ws": ws,
            }
        )

    res = run_bass_kernel_spmd(nc, in_maps, core_ids=list(range(N_CORES)))

    out_mv = np.concatenate(
        [res.results[c]["omv"].reshape(npc * STRIDE, 16, 8) for c in range(N_CORES)],
        axis=0,
    )
    out_s = np.concatenate(
        [res.results[c]["os"].reshape(npc * STRIDE, 32) for c in range(N_CORES)],
        axis=0,
    )
    return out_mv, out_s


# revision 11
# speedup vs baseline: 1.4094x; 1.4094x over previous
"""Trainium2 Bass kernel for EquivariantBallUnpooling.

Computation (per parent node n with stride=8 children):
  rel   = child_pos.reshape(n,8,3) - pos[:,None,:]
  cat_mv = [x_mv (16ch) ; embed_translation(rel) (8ch)]        # [n,24,8]
  cat_s  = [x_s (32) ; ||rel|| (8)]                            # [n,40]
  y_mv[n,o,i] = sum_c cat_mv[n,c,i] * W_mv[grade(i)][o,c]      # o in 0..127
  y_s = cat_s @ W_s.T + b_s                                    # [n,256]
  out_mv = child_x_mv + y_mv.reshape(n*8,16,8)
  out_s  = child_x_s  + y_s.reshape(n*8,32)

Sharding: data-parallel over parents across 8 cores; children stay with
their parent (blocks of stride). Weights replicated.

Per-core device kernel layout strategy: parents on the 128 SBUF
partitions, features on the free dim. Each parent's 8 children are
contiguous rows in DRAM, so child tensors load as [128, 8*feat]
contiguous lines. Activations are transposed on-chip via the PE
(contraction dim -> partitions) and the grade-wise equivariant linear
becomes 11 small matmuls + 1 for the scalar channel (bias folded in as
an extra constant-1 row).
"""

import numpy as np

import concourse.bass as bass
import concourse.mybir as mybir
import concourse.tile as tile
from concourse import bacc
from concourse.bass_utils import run_bass_kernel_spmd
from concourse.masks import make_identity

F32 = mybir.dt.float32
N_CORES = 8
STRIDE = 8
N_TOTAL = 65536
N_PER_CORE = N_TOTAL // N_CORES  # 8192
GRADE = [0, 1, 1, 1, 2, 2, 2, 3]
P = 128


def build_nc(npc: int) -> bass.Bass:
    """Build the per-core Bass program for npc parents."""
    assert npc % P == 0
    ntiles = npc // P

    nc = bacc.Bacc("TRN2", target_bir_lowering=False)

    x = nc.dram_tensor("x", [npc, 128], F32, kind="ExternalInput")
    xs = nc.dram_tensor("xs", [npc, 32], F32, kind="ExternalInput")
    pos = nc.dram_tensor("pos", [npc, 3], F32, kind="ExternalInput")
    cpos = nc.dram_tensor("cpos", [npc, 24], F32, kind="ExternalInput")
    cmv = nc.dram_tensor("cmv", [npc, 1024], F32, kind="ExternalInput")
    cs = nc.dram_tensor("cs", [npc, 256], F32, kind="ExternalInput")
    w1 = nc.dram_tensor("w1", [128, 1024], F32, kind="ExternalInput")
    w2 = nc.dram_tensor("w2", [24, 384], F32, kind="ExternalInput")
    ws = nc.dram_tensor("ws", [41, 256], F32, kind="ExternalInput")
    omv = nc.dram_tensor("omv", [npc, 1024], F32, kind="ExternalOutput")
    osc = nc.dram_tensor("os", [npc, 256], F32, kind="ExternalOutput")

    with tile.TileContext(nc) as tc:
        with (
            tc.tile_pool(name="singles", bufs=1) as singles,
            tc.tile_pool(name="loads", bufs=3) as loads,
            tc.tile_pool(name="work", bufs=3) as work,
            tc.tile_pool(name="tp_psum", bufs=2, space="PSUM") as tp_psum,
            tc.tile_pool(name="mv_psum", bufs=2, space="PSUM") as mv_psum,
            tc.tile_pool(name="s_psum", bufs=2, space="PSUM") as s_psum,
        ):
            ident = singles.tile([P, P], F32)
            make_identity(nc, ident[:])
            w1_sb = singles.tile([128, 1024], F32)
            nc.sync.dma_start(out=w1_sb[:], in_=w1[:, :])
            w2_sb = singles.tile([24, 384], F32)
            nc.sync.dma_start(out=w2_sb[:], in_=w2[:, :])
            ws_sb = singles.tile([41, 256], F32)
            nc.sync.dma_start(out=ws_sb[:], in_=ws[:, :])

            for t in range(ntiles):
                r0 = t * P

                x_t = loads.tile([P, 128], F32)
                nc.sync.dma_start(out=x_t[:], in_=x[r0 : r0 + P])
                pos_t = loads.tile([P, 3], F32)
                nc.sync.dma_start(out=pos_t[:], in_=pos[r0 : r0 + P])
                cpos_t = loads.tile([P, 24], F32)
                nc.sync.dma_start(out=cpos_t[:], in_=cpos[r0 : r0 + P])
                cmv_t = loads.tile([P, 1024], F32)
                nc.sync.dma_start(out=cmv_t[:], in_=cmv[r0 : r0 + P])
                cs_t = loads.tile([P, 256], F32)
                nc.sync.dma_start(out=cs_t[:], in_=cs[r0 : r0 + P])
                cats = work.tile([P, 41], F32)
                nc.sync.dma_start(out=cats[:, 0:32], in_=xs[r0 : r0 + P])

                # rel[p, m, d] = cpos[p, m, d] - pos[p, d]
                rel_t = work.tile([P, 24], F32)
                rel_v = rel_t[:].rearrange("p (m d) -> p m d", d=3)
                cpos_v = cpos_t[:].rearrange("p (m d) -> p m d", d=3)
                for d in range(3):
                    nc.vector.tensor_scalar_sub(
                        out=rel_v[:, :, d],
                        in0=cpos_v[:, :, d],
                        scalar1=pos_t[:, d : d + 1],
                    )
                # ||rel||: square, reduce over d, sqrt into cat_s cols 32:40
                sq_t = work.tile([P, 24], F32)
                nc.vector.tensor_mul(sq_t[:], rel_t[:], rel_t[:])
                nrm2_t = work.tile([P, 8], F32)
                nc.vector.reduce_sum(
                    out=nrm2_t[:],
                    in_=sq_t[:].rearrange("p (m d) -> p m d", d=3),
                    axis=mybir.AxisListType.X,
                )
                nc.scalar.activation(
                    cats[:, 32:40], nrm2_t[:], mybir.ActivationFunctionType.Sqrt
                )
                nc.vector.memset(cats[:, 40:41], 1.0)

                # Transposes: contraction dims onto partitions, natural row
                # order (xT rows = c*8+i, relT rows = m*3+d); the matching
                # row permutation is baked into the host-side weight layout.
                xt_ps = tp_psum.tile([P, P], F32, tag="tp")
                nc.tensor.transpose(xt_ps[:], x_t[:], ident[:])
                xT_sb = work.tile([P, P], F32)
                nc.scalar.copy(xT_sb[:], xt_ps[:])

                relT_ps = tp_psum.tile([24, P], F32, tag="tp")
                nc.tensor.transpose(relT_ps[:], rel_t[:], ident[:])
                relT_sb = work.tile([24, P], F32)
                nc.vector.tensor_copy(relT_sb[:], relT_ps[:])

                catsT_ps = tp_psum.tile([41, P], F32, tag="tp")
                nc.tensor.transpose(catsT_ps[:], cats[:], ident[:])
                catsT_sb = work.tile([41, P], F32)
                nc.vector.tensor_copy(catsT_sb[:], catsT_ps[:])

                # Grade-wise equivariant linear. Weights are block-diagonal
                # zero-padded so every matmul contracts the full transposed
                # tile from partition 0 (HW: operand base must be 0/32/64)
                # and the stationary operand is shared across components.
                # Wide N (512/384) amortizes the per-matmul LDWEIGHTS +
                # issue overhead: 3 matmuls instead of 11.
                mv_ps = mv_psum.tile([P, 1024], F32)
                nc.tensor.matmul(
                    mv_ps[:, 0:512],
                    lhsT=xT_sb[:],
                    rhs=w1_sb[:, 0:512],
                    start=True,
                    stop=False,
                )
                nc.tensor.matmul(
                    mv_ps[:, 512:1024],
                    lhsT=xT_sb[:],
                    rhs=w1_sb[:, 512:1024],
                    start=True,
                    stop=True,
                )
                nc.tensor.matmul(
                    mv_ps[:, 128:512],
                    lhsT=relT_sb[:],
                    rhs=w2_sb[:],
                    start=False,
                    stop=True,
                )

                s_ps = s_psum.tile([P, 256], F32)
                nc.tensor.matmul(
                    s_ps[:], lhsT=catsT_sb[:], rhs=ws_sb[:], start=True, stop=True
                )

                # out = child + y.  mv psum holds [i, o] slabs; child rows
                # are [o, i] interleaved, so read psum with a permuted AP.
                nc.vector.tensor_add(
                    out=cmv_t[:].rearrange("p (o i) -> p o i", i=8),
                    in0=cmv_t[:].rearrange("p (o i) -> p o i", i=8),
                    in1=mv_ps[:].rearrange("p (i o) -> p o i", i=8),
                )
                nc.vector.tensor_add(out=cs_t[:], in0=cs_t[:], in1=s_ps[:])

                nc.scalar.dma_start(out=omv[r0 : r0 + P], in_=cmv_t[:])
                nc.scalar.dma_start(out=osc[r0 : r0 + P], in_=cs_t[:])

    nc.compile()
    return nc


def _prep_weights(W_mv: np.ndarray, W_s: np.ndarray, b_s: np.ndarray):
    # Block-diagonal layouts matching the on-chip transposed activations:
    # xT rows are (c*8 + i), relT rows are (m*3 + d).
    w1 = np.zeros((128, 1024), np.float32)
    w1r = w1.reshape(16, 8, 8, 128)  # [c, i_row, i_slab, o]
    w2 = np.zeros((24, 384), np.float32)
    w2r = w2.reshape(8, 3, 3, 128)  # [m, d_row, d_slab, o]
    for i in range(8):
        w1r[:, i, i, :] = W_mv[GRADE[i]][:, :16].T
    for d in range(3):
        w2r[:, d, d, :] = W_mv[GRADE[d + 1]][:, 16:24].T
    ws = np.concatenate([W_s.T, b_s[None, :]], axis=0).astype(np.float32)
    return w1, w2, ws


_NC_CACHE: dict[int, bass.Bass] = {}


def _get_nc(npc: int) -> bass.Bass:
    if npc not in _NC_CACHE:
        _NC_CACHE[npc] = build_nc(npc)
    return _NC_CACHE[npc]


def kernel(**inputs) -> tuple[np.ndarray, np.ndarray]:
    x_mv = np.ascontiguousarray(np.asarray(inputs["x_mv"], np.float32))
    x_s = np.ascontiguousarray(np.asarray(inputs["x_s"], np.float32))
    pos = np.ascontiguousarray(np.asarray(inputs["pos"], np.float32))
    child_x_mv = np.ascontiguousarray(np.asarray(inputs["child_x_mv"], np.float32))
    child_x_s = np.ascontiguousarray(np.asarray(inputs["child_x_s"], np.float32))
    child_pos = np.ascontiguousarray(np.asarray(inputs["child_pos"], np.float32))
    W_mv = np.asarray(inputs["W_mv"], np.float32)
    W_s = np.asarray(inputs["W_s"], np.float32)
    b_s = np.asarray(inputs["b_s"], np.float32)
    stride = int(np.asarray(inputs["stride"]))
    assert stride == STRIDE, stride

    n = x_mv.shape[0]
    npc = n // N_CORES
    nc = _get_nc(npc)
    w1, w2, ws = _prep_weights(W_mv, W_s, b_s)

    in_maps = []
    for c in range(N_CORES):
        rp = slice(c * npc, (c + 1) * npc)
        rc = slice(c * npc * STRIDE, (c + 1) * npc * STRIDE)
        in_maps.append(
            {
                "x": x_mv[rp].reshape(npc, 128),
                "xs": x_s[rp],
                "pos": pos[rp],
                "cpos": child_pos[rc].reshape(npc, 24),
                "cmv": child_x_mv[rc].reshape(npc, 1024),
                "cs": child_x_s[rc].reshape(npc, 256),
                "w1": w1,
                "w2": w2,
                "ws": ws,
            }
        )

    res = run_bass_kernel_spmd(nc, in_maps, core_ids=list(range(N_CORES)))

    out_mv = np.concatenate(
        [res.results[c]["omv"].reshape(npc * STRIDE, 16, 8) for c in range(N_CORES)],
        axis=0,
    )
    out_s = np.concatenate(
        [res.results[c]["os"].reshape(npc * STRIDE, 32) for c in range(N_CORES)],
        axis=0,
    )
    return out_mv, out_s


# revision 13
# speedup vs baseline: 1.8930x; 1.3431x over previous
"""Trainium2 Bass kernel for EquivariantBallUnpooling.

Computation (per parent node n with stride=8 children):
  rel   = child_pos.reshape(n,8,3) - pos[:,None,:]
  cat_mv = [x_mv (16ch) ; embed_translation(rel) (8ch)]        # [n,24,8]
  cat_s  = [x_s (32) ; ||rel|| (8)]                            # [n,40]
  y_mv[n,o,i] = sum_c cat_mv[n,c,i] * W_mv[grade(i)][o,c]      # o in 0..127
  y_s = cat_s @ W_s.T + b_s                                    # [n,256]
  out_mv = child_x_mv + y_mv.reshape(n*8,16,8)
  out_s  = child_x_s  + y_s.reshape(n*8,32)

Sharding: data-parallel over parents across 8 cores; children stay with
their parent (blocks of stride). Weights replicated.

Per-core device kernel layout strategy: parents on the 128 SBUF
partitions, features on the free dim. Each parent's 8 children are
contiguous rows in DRAM, so child tensors load as [128, 8*feat]
contiguous lines. Activations are transposed on-chip via the PE
(contraction dim -> partitions) and the grade-wise equivariant linear
becomes 11 small matmuls + 1 for the scalar channel (bias folded in as
an extra constant-1 row).
"""

import numpy as np

import concourse.bass as bass
import concourse.mybir as mybir
import concourse.tile as tile
from concourse import bacc
from concourse.bass_utils import run_bass_kernel_spmd
from concourse.masks import make_identity

F32 = mybir.dt.float32
F32R = mybir.dt.float32r
N_CORES = 8
STRIDE = 8
N_TOTAL = 65536
N_PER_CORE = N_TOTAL // N_CORES  # 8192
GRADE = [0, 1, 1, 1, 2, 2, 2, 3]
P = 128


def build_nc(npc: int) -> bass.Bass:
    """Build the per-core Bass program for npc parents."""
    assert npc % P == 0
    ntiles = npc // P

    nc = bacc.Bacc("TRN2", target_bir_lowering=False)

    x = nc.dram_tensor("x", [npc, 128], F32, kind="ExternalInput")
    xs = nc.dram_tensor("xs", [P, ntiles * 32], F32, kind="ExternalInput")
    pos = nc.dram_tensor("pos", [P, ntiles * 3], F32, kind="ExternalInput")
    cpos = nc.dram_tensor("cpos", [P, ntiles * 24], F32, kind="ExternalInput")
    cmv = nc.dram_tensor("cmv", [npc, 1024], F32, kind="ExternalInput")
    cs = nc.dram_tensor("cs", [npc, 256], F32, kind="ExternalInput")
    w1 = nc.dram_tensor("w1", [128, 1024], F32R, kind="ExternalInput")
    w2 = nc.dram_tensor("w2", [24, 384], F32R, kind="ExternalInput")
    ws = nc.dram_tensor("ws", [41, 256], F32R, kind="ExternalInput")
    omv = nc.dram_tensor("omv", [npc, 1024], F32, kind="ExternalOutput")
    osc = nc.dram_tensor("os", [npc, 256], F32, kind="ExternalOutput")

    with tile.TileContext(nc) as tc:
        with (
            tc.tile_pool(name="singles", bufs=1) as singles,
            tc.tile_pool(name="loads", bufs=4) as loads,
            tc.tile_pool(name="work", bufs=3) as work,
            tc.tile_pool(name="tp_psum", bufs=2, space="PSUM") as tp_psum,
            tc.tile_pool(name="mv_psum", bufs=2, space="PSUM") as mv_psum,
            tc.tile_pool(name="s_psum", bufs=2, space="PSUM") as s_psum,
        ):
            ident = singles.tile([P, P], F32)
            make_identity(nc, ident[:])
            w1_sb = singles.tile([128, 1024], F32R)
            nc.sync.dma_start(out=w1_sb[:], in_=w1[:, :])
            w2_sb = singles.tile([24, 384], F32R)
            nc.sync.dma_start(out=w2_sb[:], in_=w2[:, :])
            ws_sb = singles.tile([41, 256], F32R)
            nc.sync.dma_start(out=ws_sb[:], in_=ws[:, :])
            # Host-swizzled [128, ntiles*F] layouts: small per-parent vectors
            # stay SBUF-resident for the whole kernel (their per-tile DMA
            # packets would be 12-128 B — descriptor-dominated).
            pos_all = singles.tile([P, ntiles * 3], F32)
            nc.sync.dma_start(out=pos_all[:], in_=pos[:, :])
            xs_all = singles.tile([P, ntiles * 32], F32)
            nc.sync.dma_start(out=xs_all[:], in_=xs[:, :])
            cpos_all = singles.tile([P, ntiles * 24], F32)
            nc.sync.dma_start(out=cpos_all[:], in_=cpos[:, :])

            for t in range(ntiles):
                r0 = t * P

                x_t = loads.tile([P, 128], F32)
                nc.sync.dma_start(out=x_t[:], in_=x[r0 : r0 + P])
                cmv_t = loads.tile([P, 1024], F32)
                nc.sync.dma_start(out=cmv_t[:], in_=cmv[r0 : r0 + P])
                cs_t = loads.tile([P, 256], F32)
                nc.sync.dma_start(out=cs_t[:], in_=cs[r0 : r0 + P])
                cats = work.tile([P, 41], F32)
                nc.scalar.copy(cats[:, 0:32], xs_all[:, t * 32 : t * 32 + 32])
                pos_t = pos_all[:, t * 3 : t * 3 + 3]
                cpos_t = cpos_all[:, t * 24 : t * 24 + 24]

                # rel[p, m, d] = cpos[p, m, d] - pos[p, d]
                rel_t = work.tile([P, 24], F32)
                rel_v = rel_t[:].rearrange("p (m d) -> p m d", d=3)
                cpos_v = cpos_t.rearrange("p (m d) -> p m d", d=3)
                for d in range(3):
                    nc.vector.tensor_scalar_sub(
                        out=rel_v[:, :, d],
                        in0=cpos_v[:, :, d],
                        scalar1=pos_t[:, d : d + 1],
                    )
                # ||rel||: square, reduce over d, sqrt into cat_s cols 32:40
                sq_t = work.tile([P, 24], F32)
                nc.vector.tensor_mul(sq_t[:], rel_t[:], rel_t[:])
                nrm2_t = work.tile([P, 8], F32)
                nc.vector.reduce_sum(
                    out=nrm2_t[:],
                    in_=sq_t[:].rearrange("p (m d) -> p m d", d=3),
                    axis=mybir.AxisListType.X,
                )
                nc.scalar.activation(
                    cats[:, 32:40], nrm2_t[:], mybir.ActivationFunctionType.Sqrt
                )
                nc.vector.memset(cats[:, 40:41], 1.0)

                # Transposes: contraction dims onto partitions, natural row
                # order (xT rows = c*8+i, relT rows = m*3+d); the matching
                # row permutation is baked into the host-side weight layout.
                xt_ps = tp_psum.tile([P, P], F32, tag="tp")
                nc.tensor.transpose(xt_ps[:], x_t[:], ident[:])
                xT_sb = work.tile([P, P], F32R)
                nc.scalar.copy(xT_sb[:], xt_ps[:])

                relT_ps = tp_psum.tile([24, P], F32, tag="tp")
                nc.tensor.transpose(relT_ps[:], rel_t[:], ident[:])
                relT_sb = work.tile([24, P], F32R)
                nc.vector.tensor_copy(relT_sb[:], relT_ps[:])

                catsT_ps = tp_psum.tile([41, P], F32, tag="tp")
                nc.tensor.transpose(catsT_ps[:], cats[:], ident[:])
                catsT_sb = work.tile([41, P], F32R)
                nc.vector.tensor_copy(catsT_sb[:], catsT_ps[:])

                # Grade-wise equivariant linear. Weights are block-diagonal
                # zero-padded so every matmul contracts the full transposed
                # tile from partition 0 (HW: operand base must be 0/32/64)
                # and the stationary operand is shared across components.
                # Wide N (512/384) amortizes the per-matmul LDWEIGHTS +
                # issue overhead: 3 matmuls instead of 11.
                mv_ps = mv_psum.tile([P, 1024], F32)
                nc.tensor.matmul(
                    mv_ps[:, 0:512],
                    lhsT=xT_sb[:],
                    rhs=w1_sb[:, 0:512],
                    start=True,
                    stop=False,
                )
                nc.tensor.matmul(
                    mv_ps[:, 512:1024],
                    lhsT=xT_sb[:],
                    rhs=w1_sb[:, 512:1024],
                    start=True,
                    stop=True,
                )
                nc.tensor.matmul(
                    mv_ps[:, 128:512],
                    lhsT=relT_sb[:],
                    rhs=w2_sb[:],
                    start=False,
                    stop=True,
                )

                s_ps = s_psum.tile([P, 256], F32)
                nc.tensor.matmul(
                    s_ps[:],
                    lhsT=catsT_sb[:],
                    rhs=ws_sb[:],
                    start=True,
                    stop=True,
                )

                # out = child + y.  mv psum holds [i, o] slabs; child rows
                # are [o, i] interleaved, so read psum with a permuted AP.
                nc.vector.tensor_add(
                    out=cmv_t[:].rearrange("p (o i) -> p o i", i=8),
                    in0=cmv_t[:].rearrange("p (o i) -> p o i", i=8),
                    in1=mv_ps[:].rearrange("p (i o) -> p o i", i=8),
                )
                nc.vector.tensor_add(out=cs_t[:], in0=cs_t[:], in1=s_ps[:])

                nc.scalar.dma_start(out=omv[r0 : r0 + P], in_=cmv_t[:])
                nc.scalar.dma_start(out=osc[r0 : r0 + P], in_=cs_t[:])

    nc.compile()
    return nc


def _prep_weights(W_mv: np.ndarray, W_s: np.ndarray, b_s: np.ndarray):
    # Block-diagonal layouts matching the on-chip transposed activations:
    # xT rows are (c*8 + i), relT rows are (m*3 + d).
    w1 = np.zeros((128, 1024), np.float32)
    w1r = w1.reshape(16, 8, 8, 128)  # [c, i_row, i_slab, o]
    w2 = np.zeros((24, 384), np.float32)
    w2r = w2.reshape(8, 3, 3, 128)  # [m, d_row, d_slab, o]
    for i in range(8):
        w1r[:, i, i, :] = W_mv[GRADE[i]][:, :16].T
    for d in range(3):
        w2r[:, d, d, :] = W_mv[GRADE[d + 1]][:, 16:24].T
    ws = np.concatenate([W_s.T, b_s[None, :]], axis=0).astype(np.float32)
    return w1, w2, ws


_NC_CACHE: dict[int, bass.Bass] = {}


def _get_nc(npc: int) -> bass.Bass:
    if npc not in _NC_CACHE:
        _NC_CACHE[npc] = build_nc(npc)
    return _NC_CACHE[npc]


def kernel(**inputs) -> tuple[np.ndarray, np.ndarray]:
    x_mv = np.ascontiguousarray(np.asarray(inputs["x_mv"], np.float32))
    x_s = np.ascontiguousarray(np.asarray(inputs["x_s"], np.float32))
    pos = np.ascontiguousarray(np.asarray(inputs["pos"], np.float32))
    child_x_mv = np.ascontiguousarray(np.asarray(inputs["child_x_mv"], np.float32))
    child_x_s = np.ascontiguousarray(np.asarray(inputs["child_x_s"], np.float32))
    child_pos = np.ascontiguousarray(np.asarray(inputs["child_pos"], np.float32))
    W_mv = np.asarray(inputs["W_mv"], np.float32)
    W_s = np.asarray(inputs["W_s"], np.float32)
    b_s = np.asarray(inputs["b_s"], np.float32)
    stride = int(np.asarray(inputs["stride"]))
    assert stride == STRIDE, stride

    n = x_mv.shape[0]
    npc = n // N_CORES
    ntiles = npc // P
    nc = _get_nc(npc)
    w1, w2, ws = _prep_weights(W_mv, W_s, b_s)

    def swiz(a, f):
        # [npc*, f] -> [128, ntiles*f] with tile t at cols t*f:(t+1)*f
        return np.ascontiguousarray(
            a.reshape(ntiles, P, f).transpose(1, 0, 2).reshape(P, ntiles * f)
        )

    in_maps = []
    for c in range(N_CORES):
        rp = slice(c * npc, (c + 1) * npc)
        rc = slice(c * npc * STRIDE, (c + 1) * npc * STRIDE)
        in_maps.append(
            {
                "x": x_mv[rp].reshape(npc, 128),
                "xs": swiz(x_s[rp], 32),
                "pos": swiz(pos[rp], 3),
                "cpos": swiz(child_pos[rc].reshape(npc, 24), 24),
                "cmv": child_x_mv[rc].reshape(npc, 1024),
                "cs": child_x_s[rc].reshape(npc, 256),
                "w1": w1,
                "w2": w2,
                "ws": ws,
            }
        )

    res = run_bass_kernel_spmd(nc, in_maps, core_ids=list(range(N_CORES)))

    out_mv = np.concatenate(
        [res.results[c]["omv"].reshape(npc * STRIDE, 16, 8) for c in range(N_CORES)],
        axis=0,
    )
    out_s = np.concatenate(
        [res.results[c]["os"].reshape(npc * STRIDE, 32) for c in range(N_CORES)],
        axis=0,
    )
    return out_mv, out_s
